# revision 7
# baseline (speedup 1.0000x reference)
"""Trainium2 Bass kernel for nn_BallModel: 10M-step ballistic trajectory.

The reference recurrence (pos += vel*dt; vel += g*dt, recording pos) has the
closed form
    pos_i = pos0 + i*dt*vel0 + g*dt^2 * i*(i-1)/2  =  A + B*i + C*i^2
with A = pos0, B = dt*vel0 - C, C = (g*dt)*dt/2 (per component; C_x = 0).

Output is [10_000_000, 2] f32 (~80 MB), interleaved x,y.  Each of the 8 cores
produces a contiguous 2.5M-element slice (10 MB) -> memory-bound at the
per-core HBM write bandwidth (~28 us floor).

Per core the slice is computed in 39 chunks of [128 partitions x 512 cols]
(one PSUM bank).  With element index e = core_base + c*65536 + p*512 + ce,
pair index i = q + j where q = q(core,c,p) is per-partition and j = ce>>1,
comp = ce&1 alternates x/y along the columns:

    out[p, ce] = basex(q)*even + basey(q)*odd + s1(q)*j*odd + resid(ce)
    basex(q) = A_x + B_x*q
    basey(q) = A_y + B_y*q + C*q^2
    s1(q)    = B_y + 2*C*q
    resid(ce) = B_x*j on even cols, C*j^2 on odd cols

Everything is generated by ONE K=10 bf16 matmul per chunk (PE throughput is
N columns/cycle regardless of K): per-partition values live in the stationary
operand lhsT, per-column patterns in the moving operand rhs.  Values wider
than bf16's 8 mantissa bits are split into 2-3 bf16 rows (hi/lo/lo2) whose
products accumulate exactly in the fp32 PSUM accumulator, so the result is
fp32-faithful (~1e-7 rel of the f64 closed form).  The PSUM chunk is then
copied to SBUF (alternating scalar/vector engines to split the load, or
DMA'd straight from PSUM) and written out as one contiguous 256 KB DMA.
"""

import os

import ml_dtypes
import numpy as np

import concourse.bacc as bacc
import concourse.bass as bass
import concourse.mybir as mybir
from concourse.bass_utils import run_bass_kernel_spmd
from concourse.tile import TileContext

# ---- problem constants (hardcoded; kernel.py must be self-contained) ----
N_PAIRS = 10_000_000
ELEMS = 2 * N_PAIRS  # 20,000,000 interleaved f32 values
N_CORES = 8
CE = ELEMS // N_CORES  # 2,500,000 elements per core
P = 128  # partitions
COLS = 512  # one PSUM bank of f32
CHUNK = P * COLS  # 65,536 elements per matmul chunk
NCH = -(-CE // CHUNK)  # 39 chunks/core (last one padded)
K = 10  # matmul contraction rows

# fp32-rounded constants, matching the reference's fp32 parameter rounding
DT = float(np.float32(0.01))
GDT_Y = float(np.float32(np.float32(-9.81) * np.float32(0.01)))  # fp32(g_y*dt)
C_Y = GDT_Y * DT / 2.0  # i^2 coefficient for y

_bf16 = ml_dtypes.bfloat16

# exposed for test.py introspection (exec_time_ns etc.)
LAST_RESULTS = None


def _build_program() -> bass.Bass:
    # Bacc (not raw Bass): its finalize pipeline runs
    # generate_event_semaphores, which splits >1-wait sync conditions into
    # standalone event-semaphore instructions (HW allows 1 wait/instruction).
    nc = bacc.Bacc("TRN2", target_bir_lowering=False)
    lt = nc.declare_dram_parameter("lt", [K, NCH * P], mybir.dt.bfloat16, isOutput=False)
    rh = nc.declare_dram_parameter("rh", [K, COLS], mybir.dt.bfloat16, isOutput=False)
    out = nc.declare_dram_parameter("out", [NCH * P, COLS], mybir.dt.float32, isOutput=True)

    # Walrus allows at most 2 embedded sync waits per engine instruction, so
    # the dependency structure is kept deliberately sparse:
    #  - every chunk gets its own distinct SBUF output tile (39 x 256 KB =
    #    ~10 MB of the 24 MB SBUF), so copies never carry a WAR wait on an
    #    earlier output DMA;
    #  - PSUM banks must be reused, so each copy engine gets its own PSUM
    #    pool and the matmul's bank-WAR wait stays a single wait on that
    #    engine's semaphore.
    # The PSUM->SBUF copy alternates between the scalar and vector engines
    # to split the ~27 us of copy work below the ~29 us DMA floor.
    with TileContext(nc) as tc:
        with (
            tc.tile_pool(name="const", bufs=1) as cpool,
            tc.tile_pool(name="work", bufs=1) as wpool,
            tc.tile_pool(name="psum_a", bufs=4, space="PSUM") as ppool_a,
            tc.tile_pool(name="psum_b", bufs=4, space="PSUM") as ppool_b,
        ):
            lt_s = cpool.tile([K, NCH * P], mybir.dt.bfloat16)
            rh_s = cpool.tile([K, COLS], mybir.dt.bfloat16)
            nc.sync.dma_start(lt_s[:], lt[:])
            nc.sync.dma_start(rh_s[:], rh[:])

            for c in range(NCH):
                even = c % 2 == 0
                pt = (ppool_a if even else ppool_b).tile([P, COLS], mybir.dt.float32)
                nc.tensor.matmul(
                    pt[:],
                    lt_s[:, c * P : (c + 1) * P],
                    rh_s[:],
                    start=True,
                    stop=True,
                )
                ot = wpool.tile([P, COLS], mybir.dt.float32, tag=f"ot{c}")
                if even:
                    nc.scalar.copy(ot[:], pt[:])
                else:
                    nc.vector.tensor_copy(ot[:], pt[:])
                nc.sync.dma_start(out[c * P : (c + 1) * P, :], ot[:])
    nc.finalize()  # runs Bacc.compile(): reg alloc + sync-wait legalization
    return nc


def _split_bf16(x: np.ndarray, n: int):
    """Split x into n bf16 parts summing (nearly) exactly to x."""
    parts = []
    rem = np.asarray(x, dtype=np.float64).copy()
    for _ in range(n):
        p = rem.astype(_bf16)
        parts.append(p)
        rem = rem - p.astype(np.float64)
    return parts


def _host_tables(pos0: np.ndarray, vel0: np.ndarray):
    """Build per-core input tables (float64 math, cast at the end)."""
    ax, ay = float(pos0[0]), float(pos0[1])
    bx_c = DT * float(vel0[0])  # B_x (C_x = 0)
    by_c = DT * float(vel0[1]) - C_Y  # B_y

    # fixed rhs column patterns
    ce = np.arange(COLS)
    j = (ce >> 1).astype(np.float64)
    odd = (ce & 1).astype(np.float64)
    even = 1.0 - odd
    jodd = (j * odd).astype(_bf16)  # exact: j < 256
    resid = np.where(ce & 1 == 1, C_Y * j * j, bx_c * j)
    resid_hi, resid_lo = _split_bf16(resid, 2)
    rh_np = np.stack(
        [
            jodd,
            jodd,
            resid_hi,
            resid_lo,
            odd.astype(_bf16),
            odd.astype(_bf16),
            odd.astype(_bf16),
            even.astype(_bf16),
            even.astype(_bf16),
            even.astype(_bf16),
        ]
    )  # [K, COLS]

    in_maps = []
    c_idx = np.arange(NCH, dtype=np.float64)[:, None]  # [NCH, 1]
    p_idx = np.arange(P, dtype=np.float64)[None, :]  # [1, P]
    for k in range(N_CORES):
        q = k * (CE // 2) + c_idx * (CHUNK // 2) + p_idx * (COLS // 2)  # [NCH, P]
        s1_hi, s1_lo = _split_bf16(by_c + 2.0 * C_Y * q, 2)
        ones = np.ones_like(s1_hi)
        by3 = _split_bf16(ay + by_c * q + C_Y * q * q, 3)
        bx3 = _split_bf16(ax + bx_c * q, 3)
        rows = [s1_hi, s1_lo, ones, ones] + by3 + bx3
        lt_np = np.stack([r.reshape(-1) for r in rows])  # [K, NCH*P]
        in_maps.append({"lt": lt_np, "rh": rh_np})
    return in_maps


def kernel(ball_mass, ball_initial_position, ball_initial_velocity) -> np.ndarray:
    global LAST_RESULTS
    pos0 = np.asarray(ball_initial_position, dtype=np.float32)
    vel0 = np.asarray(ball_initial_velocity, dtype=np.float32)

    nc = _build_program()
    in_maps = _host_tables(pos0, vel0)
    res = run_bass_kernel_spmd(nc, in_maps, core_ids=list(range(N_CORES)))
    LAST_RESULTS = res

    parts = [
        np.asarray(r["out"], dtype=np.float32).reshape(-1)[:CE] for r in res.results
    ]
    return np.concatenate(parts).reshape(N_PAIRS, 2)


if __name__ == "__main__":
    pos0 = np.load("/tmp/pos0.npy") if os.path.exists("/tmp/pos0.npy") else np.array(
        [-1.866805, -0.25733662], np.float32
    )
    vel0 = np.load("/tmp/vel0.npy") if os.path.exists("/tmp/vel0.npy") else np.array(
        [-0.847358, -1.5444987], np.float32
    )
    outv = kernel(np.ones(()), pos0, vel0)
    i = np.arange(N_PAIRS, dtype=np.float64)[:, None]
    closed = (
        pos0.astype(np.float64)
        + i * DT * vel0.astype(np.float64)
        + np.array([0.0, GDT_Y * DT]) * i * (i - 1) / 2.0
    )
    err = np.abs(outv - closed)
    denom = np.maximum(np.abs(closed), 1e-12)
    print("closed-form maxabs-ratio rel err:", err.max() / np.abs(closed).max())
    print("closed-form max elementwise rel err:", (err / denom).max())
